# revision 4
# baseline (speedup 1.0000x reference)
"""Trainium2 Bass kernel for SegmentationAugmentation (3D affine grid_sample, trilinear, border).

Contract: kernel(input_g, label_g, transform) -> (aug_inp f32 [8,1,128,128,128],
                                                  aug_lab bool [8,1,128,128,128])

Math (swapaxes folded into index bookkeeping; all spatial dims 128):

  out[b,c,i,j,k] = trilinear sample of input_g[b,c,:,:,:] at
      p-axis: U(i,j) = clip(64*(a00*xn(i)+a01*xn(j)+a03)+63.5, 0, 127)
      q-axis: V(i,j) = clip(64*(a10*xn(i)+a11*xn(j)+a13)+63.5, 0, 127)
      r-axis: W(k)   = clip(64*(a22*xn(k)+a23)+63.5, 0, 127)
  with xn(t) = (2t+1)/128 - 1, theta = transform[:3].  Relies on the
  generator's z-rotation structure (theta[0:2,2]==0, theta[2,0:2]==0); a
  pure-host fallback handles arbitrary transforms.

Device pipeline, data parallel over batch (core b handles batch b; each core
processes BOTH its image and label volume in one fused program):

  Phase 1 (dense): load volume chunks (f32), convert to f16 on the scalar
  engine, z-interp on DVE via run-segmented staircase slices (f16, 2x rate),
  then DMA the z-interped rows into an interleaved pair layout in DRAM:
     ZPI[r = p*128+q] = [Z0(p,q) | Z0(p+1,q) | Z1(p,q) | Z1(p+1,q)]  (1 KiB)
  (Z0 = image, Z1 = label; each row is 128 f16 k-values.)

  Phase 2 (gather): for each output point (i,j), ONE dma_gather descriptor of
  2 KiB at entry r=(p0*128+q0) fetches entries r,r+1 = all four bilinear
  corners of BOTH volumes.  DVE combines with per-point f16 weights
  (broadcast along k), accumulating the final add into f32; scalar engine
  writes outputs.

Label output is returned as f32 and thresholded on host, with voxels within
FIX_EPS of 0.5 recomputed in the reference's exact f32 arithmetic order.
"""
import numpy as np

N = 128
NROWS = N * N            # 16384 (p,q) rows per volume
NIDX = 1024              # gather indices (output points) per dma_gather call
GPC = NIDX // 128        # 8 element groups per partition per call
NCALLS = NROWS // NIDX   # 16 gather calls per rep
COLS = NIDX // 16        # idx table columns per call
ELEM = 1024              # gathered f16 elems per descriptor (= 2 ZPI entries)
ESTEP = 512              # f16 elems per ZPI entry (gather elem_step)
CH = 16                  # 128-row groups per load chunk
NGRP = NROWS // N // CH  # 8 chunks per volume
ZG = 129                 # zt groups (128 data + 1 zero pad)
FIX_EPS = 8e-3           # |label-0.5| below this -> exact host recompute

_CACHE = {}


def _mkap(pairs):
    import bass_rust
    return bass_rust.VecI64Pair([tuple(p) for p in pairs])


def _host_tables(theta):
    """All transform-derived tables, computed in float64 from f32 theta."""
    th = theta.astype(np.float64)
    t = np.arange(N, dtype=np.float64)
    xn = (2.0 * t + 1.0) / N - 1.0

    U = np.clip(64.0 * (th[0, 0] * xn[:, None] + th[0, 1] * xn[None, :] + th[0, 3]) + 63.5, 0.0, 127.0)
    V = np.clip(64.0 * (th[1, 0] * xn[:, None] + th[1, 1] * xn[None, :] + th[1, 3]) + 63.5, 0.0, 127.0)
    W = np.clip(64.0 * (th[2, 2] * xn + th[2, 3]) + 63.5, 0.0, 127.0)

    p0 = np.floor(U).astype(np.int64)
    q0 = np.floor(V).astype(np.int64)
    r0 = np.floor(W).astype(np.int64)
    fu = (U - p0).astype(np.float32)
    fv = (V - q0).astype(np.float32)
    fw = (W - r0).astype(np.float32)
    r1 = np.minimum(r0 + 1, N - 1)

    idxA = (p0 * 128 + q0).astype(np.int16)          # [i,j] ZPI entry index
    w00 = ((1 - fu) * (1 - fv)).astype(np.float32)
    w10 = (fu * (1 - fv)).astype(np.float32)
    w01 = ((1 - fu) * fv).astype(np.float32)
    w11 = (fu * fv).astype(np.float32)

    # z-run decomposition: maximal segments where both r0 and r1 step by a
    # constant d in {-1,0,1}
    runs = []
    k = 0
    while k < N:
        step = 0
        if k + 1 < N:
            d = int(r0[k + 1] - r0[k])
            if d == int(r1[k + 1] - r1[k]) and d in (-1, 0, 1):
                step = d
        ln = 1
        while (k + ln < N
               and int(r0[k + ln] - r0[k]) == step * ln
               and int(r1[k + ln] - r1[k]) == step * ln):
            ln += 1
        runs.append((k, ln, int(r0[k]), int(r1[k]), step))
        k += ln

    return dict(idxA=idxA, w00=w00, w01=w01, w10=w10, w11=w11, fw=fw, runs=runs)


def _pack_idxs(idx_flat):
    """int16 dma_gather index layout: element i at [i%16, i//16], replicated to 128 partitions."""
    t = idx_flat.reshape(-1, 16).T.astype(np.int16)  # [16, n/16]
    return np.ascontiguousarray(np.tile(t, (8, 1)))  # [128, n/16]


def _build_program(tables, reps=1):
    """Raw-Bass (explicit semaphore) program; see module docstring for the
    pipeline.  All cross-engine waits are standalone wait_ge instructions.

    Engine streams:
      sync   (SP HWDGE):  const loads, volume loads, ZPI stream writes
      scalar (ACT):       f32->f16 chunk converts; output writes (HWDGE)
      vector (DVE):       z-interp (phase 1), 4-corner combine (phase 2)
      gpsimd (SWDGE):     one dma_gather per 1024 output points
    """
    import concourse.bass as bass
    from concourse import bacc, mybir

    runs = tables["runs"]
    f32 = mybir.dt.float32
    f16 = mybir.dt.float16
    i16 = mybir.dt.int16

    nc = bacc.Bacc("TRN2", target_bir_lowering=False, debug=False, num_devices=8)

    vol_in = [nc.dram_tensor(f"vol{v}", [NROWS, N], f32, kind="ExternalInput") for v in range(2)]
    idx_dram = nc.dram_tensor("idxA", [128, NROWS // 16], i16, kind="ExternalInput")
    wts = nc.dram_tensor("wts", [4, 128, 128], f16, kind="ExternalInput")
    fwrep = nc.dram_tensor("fwrep", [2, 128, 128], f16, kind="ExternalInput")
    vol_out = [nc.dram_tensor(f"out{v}", [NROWS, N], f32, kind="ExternalOutput") for v in range(2)]
    zpi = nc.dram_tensor("zpi", [NROWS + 1, ESTEP], f16, kind="Internal")

    AP = bass.AP

    idx_t = nc.alloc_sbuf_tensor("idx_t", [128, NROWS // 16], i16)
    w_t = [nc.alloc_sbuf_tensor(f"w{c}_t", [128, 128], f16) for c in range(4)]
    fw_t = [nc.alloc_sbuf_tensor(f"fw{c}_t", [128, 128], f16) for c in range(2)]
    vt32 = [nc.alloc_sbuf_tensor(f"vt32_{s}", [128, CH * N], f32) for s in range(2)]
    vt16 = [nc.alloc_sbuf_tensor(f"vt16_{s}", [128, CH * N], f16) for s in range(2)]
    zt = [nc.alloc_sbuf_tensor(f"zt{v}", [128, ZG * N], f16) for v in range(2)]
    ztmp = nc.alloc_sbuf_tensor("ztmp", [128, CH * N], f16)
    At = [nc.alloc_sbuf_tensor(f"At{s}", [128, GPC * ELEM], f16) for s in range(2)]
    mt = [nc.alloc_sbuf_tensor(f"m{s}", [128, GPC * N], f16) for s in range(8)]
    acc = [[nc.alloc_sbuf_tensor(f"acc{v}_{s}", [128, GPC * N], f32) for s in range(2)]
           for v in range(2)]

    nrows_ap = NROWS  # gather element at entry r reads entries r, r+1; r <= 16383

    ZGN = ZG * N
    NC_ = NCALLS

    from contextlib import ExitStack
    with ExitStack() as _sctx:
        block = _sctx.enter_context(nc.Block())
        s_idx = _sctx.enter_context(nc.semaphore("s_idx"))
        s_wf = _sctx.enter_context(nc.semaphore("s_wf"))
        s_mz = _sctx.enter_context(nc.semaphore("s_mz"))
        s_l = [_sctx.enter_context(nc.semaphore(f"s_l{p}")) for p in range(2)]
        s_cv = _sctx.enter_context(nc.semaphore("s_cv"))
        s_z = _sctx.enter_context(nc.semaphore("s_z"))
        s_zw = _sctx.enter_context(nc.semaphore("s_zw"))
        s_g = [_sctx.enter_context(nc.semaphore(f"s_g{p}")) for p in range(2)]
        s_c = _sctx.enter_context(nc.semaphore("s_c"))
        s_o = [_sctx.enter_context(nc.semaphore(f"s_o{p}")) for p in range(2)]
        s_v = _sctx.enter_context(nc.semaphore("s_v"))

        @block.sync
        def _(sync):
            sync.dma_start(idx_t.ap(), idx_dram.ap()).then_inc(s_idx, 16)
            for c in range(4):
                sync.dma_start(w_t[c].ap(), AP(wts, c * 128 * 128, [[128, 128], [1, 128]])).then_inc(s_wf, 16)
            for c in range(2):
                sync.dma_start(fw_t[c].ap(), AP(fwrep, c * 128 * 128, [[128, 128], [1, 128]])).then_inc(s_wf, 16)
            # one-time: zero ZPI entry 16384 (read by gathers at r=16383)
            sync.wait_ge(s_mz, 1)
            sync.dma_start(
                AP(zpi, NROWS * ESTEP, [[128, 4], [1, 128]]),
                AP(zt[0], 128 * N, [[ZGN, 4], [1, 128]]),
            ).then_inc(s_zw, 16)
            for r in range(reps):
                for v in range(2):
                    for g in range(NGRP):
                        t = (r * 2 + v) * NGRP + g
                        if t >= 2:
                            sync.wait_ge(s_cv, t - 1)  # WAR vt32 vs convert
                        sync.dma_start(
                            AP(vt32[t % 2], 0, [[CH * N, 128], [N, CH], [1, N]]),
                            AP(vol_in[v], g * CH * 128 * N, [[N, 128], [128 * N, CH], [1, N]]),
                        ).then_inc(s_l[t % 2], 16)
                if r >= 1:
                    sync.wait_ge(s_c, NC_ * r)  # WAR zpi vs prev-rep gathers
                for v in range(2):
                    sync.wait_ge(s_z, (r * 2 + v + 1) * NGRP)
                    for h in range(2):
                        sync.dma_start(
                            AP(zpi, v * 256 + h * 128, [[ESTEP, 128], [128 * ESTEP, 128], [1, N]]),
                            AP(zt[v], h * N, [[ZGN, 128], [N, 128], [1, N]]),
                        ).then_inc(s_zw, 16)
            sync.wait_ge(s_o[0], 32 * ((NC_ * reps + 1) // 2))
            sync.wait_ge(s_o[1], 32 * (NC_ * reps // 2))

        @block.scalar
        def _(scalar):
            for r in range(reps):
                for v in range(2):
                    for g in range(NGRP):
                        t = (r * 2 + v) * NGRP + g
                        scalar.wait_ge(s_l[t % 2], 16 * (t // 2 + 1))
                        if t >= 2:
                            scalar.wait_ge(s_z, t - 1)  # WAR vt16 vs z-interp
                        scalar.copy(vt16[t % 2].ap(), vt32[t % 2].ap()).then_inc(s_cv, 1)
                for c in range(NC_):
                    gc = r * NC_ + c
                    scalar.wait_ge(s_c, gc + 1)
                    for v in range(2):
                        scalar.dma_start(
                            AP(vol_out[v], c * NIDX * N, [[N, 128], [128 * N, GPC], [1, N]]),
                            AP(acc[v][gc % 2], 0, [[GPC * N, 128], [N, GPC], [1, N]]),
                        ).then_inc(s_o[gc % 2], 16)

        @block.gpsimd
        def _(gpsimd):
            nreg = gpsimd.to_reg(NIDX)
            gpsimd.wait_ge(s_idx, 16)
            sv = AP(zpi, 0, [[ESTEP, nrows_ap], [1, ELEM]])
            for r in range(reps):
                gpsimd.wait_ge(s_zw, 16 + 64 * (r + 1))
                for c in range(NC_):
                    gc = r * NC_ + c
                    if gc >= 2:
                        gpsimd.wait_ge(s_c, gc - 1)  # WAR At vs combine
                    gpsimd.dma_gather(
                        AP(At[gc % 2], 0, [[GPC * ELEM, 128], [ELEM, GPC], [1, ELEM]]),
                        sv,
                        AP(idx_t, c * COLS, [[NROWS // 16, 128], [1, COLS]]),
                        NIDX, nreg, ELEM, elem_step=ESTEP,
                    ).then_inc(s_g[gc % 2], 16)

        @block.vector
        def _(vector):
            mult = mybir.AluOpType.mult
            VC = [0]

            def vsync(last_ins):
                # DVE pipeline does not interlock same-engine RAW hazards
                last_ins.then_inc(s_v, 1)
                VC[0] += 1
                vector.wait_ge(s_v, VC[0])

            vector.wait_ge(s_wf, 96)
            last = None
            for v in range(2):
                last = vector.memset(AP(zt[v], 128 * N, [[ZGN, 128], [1, N]]), 0.0)
            last.then_inc(s_mz, 1)

            def zgroup(r, v, g):
                t = (r * 2 + v) * NGRP + g
                if t >= 1:
                    vector.wait_ge(s_z, t)  # WAR ztmp/pipeline drain
                vector.wait_ge(s_cv, t + 1)
                if r >= 1:
                    # WAR zt[v] vs prev rep's ZPI streams
                    vector.wait_ge(s_zw, 16 + 64 * (r - 1) + 32 * (v + 1))
                s = vt16[t % 2]
                last_ins = None
                for (ks, ln, r0s, r1s, st) in runs:
                    zdst = AP(zt[v], g * CH * N + ks, [[ZGN, 128], [N, CH], [1, ln]])
                    tdst = AP(ztmp, ks, [[CH * N, 128], [N, CH], [1, ln]])
                    v0 = AP(s, r0s, [[CH * N, 128], [N, CH], [st, ln]])
                    v1 = AP(s, r1s, [[CH * N, 128], [N, CH], [st, ln]])
                    f0 = AP(fw_t[0], ks, [[128, 128], [0, CH], [1, ln]])
                    f1 = AP(fw_t[1], ks, [[128, 128], [0, CH], [1, ln]])
                    vector.tensor_tensor(zdst, v0, f0, mult)
                    last_ins = vector.tensor_tensor(tdst, v1, f1, mult)
                vsync(last_ins)
                for (ks, ln, r0s, r1s, st) in runs:
                    zdst = AP(zt[v], g * CH * N + ks, [[ZGN, 128], [N, CH], [1, ln]])
                    tdst = AP(ztmp, ks, [[CH * N, 128], [N, CH], [1, ln]])
                    last_ins = vector.tensor_add(zdst, zdst, tdst)
                last_ins.then_inc(s_z, 1)

            def combine(r, c):
                gc = r * NC_ + c
                if gc >= 1:
                    vector.wait_ge(s_c, gc)  # WAR mt vs prev combine
                vector.wait_ge(s_g[gc % 2], 16 * (gc // 2 + 1))
                if gc >= 2:
                    vector.wait_ge(s_o[gc % 2], 32 * (gc // 2))  # WAR acc
                A = At[gc % 2]
                shp = [[GPC * ELEM, 128], [ELEM, GPC], [1, N]]
                oshp = [[GPC * N, 128], [N, GPC], [1, N]]

                def wb(ci):
                    return AP(w_t[ci], c * GPC, [[128, 128], [1, GPC], [0, N]])
                maps = [AP(m, 0, oshp) for m in mt]
                # corner offsets within a gathered element (f16 elems):
                #   vol v: (p0,q0)=v*256, (p1,q0)=v*256+128,
                #          (p0,q1)=v*256+512, (p1,q1)=v*256+640
                last_ins = None
                for v in range(2):
                    b = 4 * v
                    vector.tensor_tensor(maps[b + 0], AP(A, v * 256 + 0, shp), wb(0), mult)
                    vector.tensor_tensor(maps[b + 1], AP(A, v * 256 + 128, shp), wb(2), mult)
                    vector.tensor_tensor(maps[b + 2], AP(A, v * 256 + 512, shp), wb(1), mult)
                    last_ins = vector.tensor_tensor(maps[b + 3], AP(A, v * 256 + 640, shp), wb(3), mult)
                vsync(last_ins)
                for v in range(2):
                    b = 4 * v
                    vector.tensor_add(maps[b + 0], maps[b + 0], maps[b + 1])
                    last_ins = vector.tensor_add(maps[b + 2], maps[b + 2], maps[b + 3])
                vsync(last_ins)
                vector.tensor_add(AP(acc[0][gc % 2], 0, oshp), maps[0], maps[2])
                vector.tensor_add(AP(acc[1][gc % 2], 0, oshp), maps[4], maps[6]) \
                    .then_inc(s_c, 1)

            for r in range(reps):
                for v in range(2):
                    for g in range(NGRP):
                        zgroup(r, v, g)
                for c in range(NC_):
                    combine(r, c)

    nc.compile()
    return nc


def _exact_label_fixup(label_g, theta, lab_f, out_bool):
    """Recompute voxels of |lab_f - 0.5| < FIX_EPS in the reference's exact
    f32 arithmetic order (validated bit-exact against the jax reference)."""
    eps = np.float32(FIX_EPS)
    cand = np.abs(lab_f - np.float32(0.5)) < eps
    if not cand.any():
        return out_bool
    bb, ii, jj, kk = np.nonzero(cand.reshape(-1, N, N, N))
    v = _exact_reference_values(label_g, theta, bb, ii, jj, kk)
    out_bool.reshape(-1, N, N, N)[bb, ii, jj, kk] = v > np.float32(0.5)
    return out_bool


def _exact_reference_values(vol_g, theta, bb, ii, jj, kk):
    """Reference-order f32 trilinear values at selected voxels.

    Replicates: grid einsum (x*t0 + y*t1 + z*t2, left-assoc f32) + t3; unnorm;
    8-corner accumulation in (z,y,x) order with w=(wz*wy)*wx, out += v*w.
    """
    f32 = np.float32
    t = np.arange(N, dtype=f32)
    xn = ((f32(2.0) * t + f32(1.0)) / f32(N) - f32(1.0)).astype(f32)
    th = theta.astype(f32)

    x = xn[ii]; y = xn[jj]; z = xn[kk]

    # f32 fma via f64 (exact up to negligible double-rounding corner cases)
    def fma32(a, b, c):
        return (np.float64(a) * np.float64(b) + c.astype(np.float64)).astype(f32)

    # grid components — XLA CPU lowers the einsum as an FMA chain (verified
    # bit-exact): fma(z, t2, fma(y, t1, x*t0)) + t3
    def comp(r):
        a = fma32(y, th[r, 1], (x * th[r, 0]).astype(f32))
        a = fma32(z, th[r, 2], a)
        return (a + th[r, 3]).astype(f32)
    gx, gy, gz = comp(0), comp(1), comp(2)

    def unnorm(c):
        return np.clip(((c + f32(1.0)) * f32(N) - f32(1.0)) * f32(0.5), f32(0.0), f32(N - 1))
    ux, uy, uz = unnorm(gx), unnorm(gy), unnorm(gz)
    x0 = np.floor(ux); y0 = np.floor(uy); z0 = np.floor(uz)
    fx = (ux - x0).astype(f32); fy = (uy - y0).astype(f32); fz = (uz - z0).astype(f32)
    x0i = x0.astype(np.int64); y0i = y0.astype(np.int64); z0i = z0.astype(np.int64)
    x1i = np.minimum(x0i + 1, N - 1); y1i = np.minimum(y0i + 1, N - 1); z1i = np.minimum(z0i + 1, N - 1)

    vol = vol_g.reshape(-1, N, N, N)
    out = np.zeros(bb.shape, f32)
    one = f32(1.0)
    for zi, wz in ((z0i, (one - fz).astype(f32)), (z1i, fz)):
        for yi, wy in ((y0i, (one - fy).astype(f32)), (y1i, fy)):
            for xi, wx in ((x0i, (one - fx).astype(f32)), (x1i, fx)):
                # inp[b, c, zi, yi, xi] in transposed space == vol[b, xi, yi, zi]
                vals = vol[bb, xi, yi, zi]
                w = ((wz * wy).astype(f32) * wx).astype(f32)
                out = (out + (vals * w).astype(f32)).astype(f32)
    return out


def _host_fallback(input_g, label_g, transform):
    """Arbitrary-transform fallback: full reference computation on host."""
    bb, ii, jj, kk = np.meshgrid(np.arange(8), np.arange(N), np.arange(N), np.arange(N), indexing="ij")
    bb, ii, jj, kk = (a.reshape(-1) for a in (bb, ii, jj, kk))
    theta = transform[:3].astype(np.float32)
    aug_inp = _exact_reference_values(input_g, theta, bb, ii, jj, kk).reshape(8, 1, N, N, N)
    lab = _exact_reference_values(label_g, theta, bb, ii, jj, kk).reshape(8, 1, N, N, N)
    return aug_inp.astype(np.float32), lab > np.float32(0.5)


def _make_inputs(tables, input_g, label_g):
    idx_p = _pack_idxs(tables["idxA"].reshape(-1))
    wts = np.stack([tables[c].T.copy() for c in ("w00", "w01", "w10", "w11")]).astype(np.float16)
    fwrep = np.stack([np.tile(1.0 - tables["fw"], (128, 1)),
                      np.tile(tables["fw"], (128, 1))]).astype(np.float16)
    in_maps = []
    for b in range(8):
        in_maps.append({
            "vol0": input_g[b, 0].reshape(NROWS, N),
            "vol1": label_g[b, 0].reshape(NROWS, N),
            "idxA": idx_p, "wts": wts, "fwrep": fwrep,
        })
    return in_maps


def kernel(input_g, label_g, transform):
    input_g = np.ascontiguousarray(input_g, dtype=np.float32)
    label_g = np.ascontiguousarray(label_g, dtype=np.float32)
    transform = np.asarray(transform, dtype=np.float32)
    theta = transform[:3]

    structured = (abs(float(theta[0, 2])) < 1e-12 and abs(float(theta[1, 2])) < 1e-12
                  and abs(float(theta[2, 0])) < 1e-12 and abs(float(theta[2, 1])) < 1e-12)
    if not structured:
        return _host_fallback(input_g, label_g, transform)

    from concourse.bass_utils import run_bass_kernel_spmd

    tables = _host_tables(theta)
    key = transform.tobytes()
    if key not in _CACHE:
        _CACHE[key] = _build_program(tables)
    nc = _CACHE[key]

    in_maps = _make_inputs(tables, input_g, label_g)
    res = run_bass_kernel_spmd(nc, in_maps, core_ids=list(range(8)))

    aug_inp = np.empty((8, 1, N, N, N), np.float32)
    lab_f = np.empty((8, 1, N, N, N), np.float32)
    for b in range(8):
        aug_inp[b, 0] = res.results[b]["out0"].reshape(N, N, N)
        lab_f[b, 0] = res.results[b]["out1"].reshape(N, N, N)

    out_bool = lab_f > np.float32(0.5)
    out_bool = _exact_label_fixup(label_g, theta, lab_f, out_bool)
    return aug_inp, out_bool


# revision 6
# speedup vs baseline: 1.1197x; 1.1197x over previous
"""Trainium2 Bass kernel for SegmentationAugmentation (3D affine grid_sample, trilinear, border).

Contract: kernel(input_g, label_g, transform) -> (aug_inp f32 [8,1,128,128,128],
                                                  aug_lab bool [8,1,128,128,128])

Math (swapaxes folded into index bookkeeping; all spatial dims 128):

  out[b,c,i,j,k] = trilinear sample of input_g[b,c,:,:,:] at
      p-axis: U(i,j) = clip(64*(a00*xn(i)+a01*xn(j)+a03)+63.5, 0, 127)
      q-axis: V(i,j) = clip(64*(a10*xn(i)+a11*xn(j)+a13)+63.5, 0, 127)
      r-axis: W(k)   = clip(64*(a22*xn(k)+a23)+63.5, 0, 127)
  with xn(t) = (2t+1)/128 - 1, theta = transform[:3].  Relies on the
  generator's z-rotation structure (theta[0:2,2]==0, theta[2,0:2]==0); a
  pure-host fallback handles arbitrary transforms.

Device pipeline, data parallel over batch (core b handles batch b; each core
processes BOTH its image and label volume in one fused f16 program):

  Phase 1 (dense): load pre-transposed f16 volume chunks, z-interp on DVE via
  run-segmented staircase slices (f16, 2x perf mode), then DMA the z-interped
  rows into an interleaved pair layout in DRAM:
     ZPI[r = p*128+q] = [Z0(p,q) | Z0(p+1,q) | Z1(p,q) | Z1(p+1,q)]  (1 KiB)
  (Z0 = image, Z1 = label; each row is 128 f16 k-values.)

  Phase 2 (gather): for each output point (i,j), ONE dma_gather descriptor of
  2 KiB at entry r=(p0*128+q0) fetches entries r,r+1 = all four bilinear
  corners of BOTH volumes.  DVE combines with k-replicated f16 weight tiles
  (streamed from DRAM per call so every operand keeps innermost stride 1 and
  2-byte dtype -> DVE 2x perf mode); one 512B-descriptor DMA per call writes
  the interleaved f16 outputs of both volumes.

Host converts the f16 outputs to f32 / bool; label voxels within FIX_EPS of
0.5 are recomputed in the reference's exact f32 arithmetic order.
"""
import numpy as np

N = 128
NROWS = N * N            # 16384 (p,q) rows per volume
NIDX = 1024              # gather indices (output points) per dma_gather call
GPC = NIDX // 128        # 8 element groups per partition per call
NCALLS = NROWS // NIDX   # 16 gather calls per rep
COLS = NIDX // 16        # idx table columns per call
ELEM = 1024              # gathered f16 elems per descriptor (= 2 ZPI entries)
ESTEP = 512              # f16 elems per ZPI entry (gather elem_step)
CH = 16                  # 128-row groups per load chunk
NGRP = NROWS // N // CH  # 8 chunks per volume
ZG = 129                 # zt groups (128 data + 1 zero pad)
FIX_EPS = 8e-3           # |label-0.5| below this -> exact host recompute

_CACHE = {}


def _mkap(pairs):
    import bass_rust
    return bass_rust.VecI64Pair([tuple(p) for p in pairs])


def _host_tables(theta):
    """All transform-derived tables, computed in float64 from f32 theta."""
    th = theta.astype(np.float64)
    t = np.arange(N, dtype=np.float64)
    xn = (2.0 * t + 1.0) / N - 1.0

    U = np.clip(64.0 * (th[0, 0] * xn[:, None] + th[0, 1] * xn[None, :] + th[0, 3]) + 63.5, 0.0, 127.0)
    V = np.clip(64.0 * (th[1, 0] * xn[:, None] + th[1, 1] * xn[None, :] + th[1, 3]) + 63.5, 0.0, 127.0)
    W = np.clip(64.0 * (th[2, 2] * xn + th[2, 3]) + 63.5, 0.0, 127.0)

    p0 = np.floor(U).astype(np.int64)
    q0 = np.floor(V).astype(np.int64)
    r0 = np.floor(W).astype(np.int64)
    fu = (U - p0).astype(np.float32)
    fv = (V - q0).astype(np.float32)
    fw = (W - r0).astype(np.float32)
    r1 = np.minimum(r0 + 1, N - 1)

    idxA = (p0 * 128 + q0).astype(np.int16)          # [i,j] ZPI entry index
    w00 = ((1 - fu) * (1 - fv)).astype(np.float32)
    w10 = (fu * (1 - fv)).astype(np.float32)
    w01 = ((1 - fu) * fv).astype(np.float32)
    w11 = (fu * fv).astype(np.float32)

    # z-run decomposition: maximal segments where both r0 and r1 step by a
    # constant d in {-1,0,1}
    runs = []
    k = 0
    while k < N:
        step = 0
        if k + 1 < N:
            d = int(r0[k + 1] - r0[k])
            if d == int(r1[k + 1] - r1[k]) and d in (-1, 0, 1):
                step = d
        ln = 1
        while (k + ln < N
               and int(r0[k + ln] - r0[k]) == step * ln
               and int(r1[k + ln] - r1[k]) == step * ln):
            ln += 1
        runs.append((k, ln, int(r0[k]), int(r1[k]), step))
        k += ln

    return dict(idxA=idxA, w00=w00, w01=w01, w10=w10, w11=w11, fw=fw, runs=runs)


def _pack_idxs(idx_flat):
    """int16 dma_gather index layout: element i at [i%16, i//16], replicated to 128 partitions."""
    t = idx_flat.reshape(-1, 16).T.astype(np.int16)  # [16, n/16]
    return np.ascontiguousarray(np.tile(t, (8, 1)))  # [128, n/16]


def _build_program(tables, reps=1):
    """Raw-Bass (explicit semaphore) program; see module docstring for the
    pipeline.  All cross-engine waits are standalone wait_ge instructions.

    Engine streams:
      sync   (SP HWDGE):  const/volume/weight-tile loads, ZPI stream writes
      scalar (ACT HWDGE): interleaved output writes
      vector (DVE):       z-interp (phase 1), 4-corner combine (phase 2)
      gpsimd (SWDGE):     one dma_gather per 1024 output points
    """
    import concourse.bass as bass
    from concourse import bacc, mybir

    runs = tables["runs"]
    f16 = mybir.dt.float16
    i16 = mybir.dt.int16

    nc = bacc.Bacc("TRN2", target_bir_lowering=False, debug=False, num_devices=8)

    vol_in = [nc.dram_tensor(f"vol{v}", [128, NROWS], f16, kind="ExternalInput") for v in range(2)]
    idx_dram = nc.dram_tensor("idxA", [128, NROWS // 16], i16, kind="ExternalInput")
    wtile = nc.dram_tensor("wtile", [4, NCALLS, 128, GPC * N], f16, kind="ExternalInput")
    fwrep = nc.dram_tensor("fwrep", [2, 128, 128], f16, kind="ExternalInput")
    out_i = nc.dram_tensor("outI", [NROWS, 256], f16, kind="ExternalOutput")
    zpi = nc.dram_tensor("zpi", [NROWS + 1, ESTEP], f16, kind="Internal")

    AP = bass.AP

    idx_t = nc.alloc_sbuf_tensor("idx_t", [128, NROWS // 16], i16)
    fw_t = [nc.alloc_sbuf_tensor(f"fw{c}_t", [128, 128], f16) for c in range(2)]
    wt_sb = [[nc.alloc_sbuf_tensor(f"wt{c}_{s}", [128, GPC * N], f16) for s in range(2)]
             for c in range(4)]
    vt16 = [nc.alloc_sbuf_tensor(f"vt16_{s}", [128, CH * N], f16) for s in range(2)]
    zt = [nc.alloc_sbuf_tensor(f"zt{v}", [128, ZG * N], f16) for v in range(2)]
    ztmp = nc.alloc_sbuf_tensor("ztmp", [128, CH * N], f16)
    At = [nc.alloc_sbuf_tensor(f"At{s}", [128, GPC * ELEM], f16) for s in range(2)]
    mt = [nc.alloc_sbuf_tensor(f"m{s}", [128, GPC * N], f16) for s in range(8)]
    accb = [nc.alloc_sbuf_tensor(f"accb{s}", [128, GPC * 256], f16) for s in range(2)]

    nrows_ap = NROWS  # gather element at entry r reads entries r, r+1; r <= 16383

    ZGN = ZG * N
    NC_ = NCALLS

    from contextlib import ExitStack
    with ExitStack() as _sctx:
        block = _sctx.enter_context(nc.Block())
        s_idx = _sctx.enter_context(nc.semaphore("s_idx"))
        s_wf = _sctx.enter_context(nc.semaphore("s_wf"))
        s_mz = _sctx.enter_context(nc.semaphore("s_mz"))
        s_l = [_sctx.enter_context(nc.semaphore(f"s_l{p}")) for p in range(2)]
        s_wl = [_sctx.enter_context(nc.semaphore(f"s_wl{p}")) for p in range(2)]
        s_z = _sctx.enter_context(nc.semaphore("s_z"))
        s_zw = _sctx.enter_context(nc.semaphore("s_zw"))
        s_g = [_sctx.enter_context(nc.semaphore(f"s_g{p}")) for p in range(2)]
        s_c = _sctx.enter_context(nc.semaphore("s_c"))
        s_o = [_sctx.enter_context(nc.semaphore(f"s_o{p}")) for p in range(2)]
        s_v = _sctx.enter_context(nc.semaphore("s_v"))

        @block.sync
        def _(sync):
            sync.dma_start(idx_t.ap(), idx_dram.ap()).then_inc(s_idx, 16)
            for c in range(2):
                sync.dma_start(fw_t[c].ap(), AP(fwrep, c * 128 * 128, [[128, 128], [1, 128]])).then_inc(s_wf, 16)
            # one-time: zero ZPI entry 16384 (read by gathers at r=16383)
            sync.wait_ge(s_mz, 1)
            sync.dma_start(
                AP(zpi, NROWS * ESTEP, [[128, 4], [1, 128]]),
                AP(zt[0], 128 * N, [[ZGN, 4], [1, 128]]),
            ).then_inc(s_zw, 16)
            for r in range(reps):
                for v in range(2):
                    for g in range(NGRP):
                        t = (r * 2 + v) * NGRP + g
                        if t >= 2:
                            sync.wait_ge(s_z, t - 1)  # WAR vt16 vs z-interp
                        sync.dma_start(
                            AP(vt16[t % 2], 0, [[CH * N, 128], [1, CH * N]]),
                            AP(vol_in[v], g * CH * N, [[NROWS, 128], [1, CH * N]]),
                        ).then_inc(s_l[t % 2], 16)
                if r >= 1:
                    sync.wait_ge(s_c, NC_ * r)  # WAR zpi vs prev-rep gathers
                for v in range(2):
                    sync.wait_ge(s_z, (r * 2 + v + 1) * NGRP)
                    for h in range(2):
                        sync.dma_start(
                            AP(zpi, v * 256 + h * 128, [[ESTEP, 128], [128 * ESTEP, 128], [1, N]]),
                            AP(zt[v], h * N, [[ZGN, 128], [N, 128], [1, N]]),
                        ).then_inc(s_zw, 16)
                # phase 2: per-call weight tiles (double buffered)
                for c in range(NC_):
                    gc = r * NC_ + c
                    if gc >= 2:
                        sync.wait_ge(s_c, gc - 1)  # WAR wt_sb vs combine
                    for ci in range(4):
                        sync.dma_start(
                            wt_sb[ci][gc % 2].ap(),
                            AP(wtile, (ci * NC_ + c) * 128 * GPC * N, [[GPC * N, 128], [1, GPC * N]]),
                        ).then_inc(s_wl[gc % 2], 16)
            sync.wait_ge(s_o[0], 16 * ((NC_ * reps + 1) // 2))
            sync.wait_ge(s_o[1], 16 * (NC_ * reps // 2))

        @block.scalar
        def _(scalar):
            for r in range(reps):
                for c in range(NC_):
                    gc = r * NC_ + c
                    scalar.wait_ge(s_c, gc + 1)
                    scalar.dma_start(
                        AP(out_i, c * NIDX * 256, [[256, 128], [128 * 256, GPC], [1, 256]]),
                        AP(accb[gc % 2], 0, [[GPC * 256, 128], [256, GPC], [1, 256]]),
                    ).then_inc(s_o[gc % 2], 16)

        @block.gpsimd
        def _(gpsimd):
            nreg = gpsimd.to_reg(NIDX)
            gpsimd.wait_ge(s_idx, 16)
            sv = AP(zpi, 0, [[ESTEP, nrows_ap], [1, ELEM]])
            for r in range(reps):
                gpsimd.wait_ge(s_zw, 16 + 64 * (r + 1))
                for c in range(NC_):
                    gc = r * NC_ + c
                    if gc >= 2:
                        gpsimd.wait_ge(s_c, gc - 1)  # WAR At vs combine
                    gpsimd.dma_gather(
                        AP(At[gc % 2], 0, [[GPC * ELEM, 128], [ELEM, GPC], [1, ELEM]]),
                        sv,
                        AP(idx_t, c * COLS, [[NROWS // 16, 128], [1, COLS]]),
                        NIDX, nreg, ELEM, elem_step=ESTEP,
                    ).then_inc(s_g[gc % 2], 16)

        @block.vector
        def _(vector):
            mult = mybir.AluOpType.mult
            VC = [0]

            def vsync(last_ins):
                # DVE pipeline does not interlock same-engine RAW hazards
                last_ins.then_inc(s_v, 1)
                VC[0] += 1
                vector.wait_ge(s_v, VC[0])

            vector.wait_ge(s_wf, 32)
            last = None
            for v in range(2):
                last = vector.memset(AP(zt[v], 128 * N, [[ZGN, 128], [1, N]]), 0.0)
            last.then_inc(s_mz, 1)

            def zgroup(r, v, g):
                t = (r * 2 + v) * NGRP + g
                if t >= 1:
                    vector.wait_ge(s_z, t)  # WAR ztmp/pipeline drain
                vector.wait_ge(s_l[t % 2], 16 * (t // 2 + 1))
                if r >= 1:
                    # WAR zt[v] vs prev rep's ZPI streams
                    vector.wait_ge(s_zw, 16 + 64 * (r - 1) + 32 * (v + 1))
                s = vt16[t % 2]
                last_ins = None
                for (ks, ln, r0s, r1s, st) in runs:
                    zdst = AP(zt[v], g * CH * N + ks, [[ZGN, 128], [N, CH], [1, ln]])
                    tdst = AP(ztmp, ks, [[CH * N, 128], [N, CH], [1, ln]])
                    v0 = AP(s, r0s, [[CH * N, 128], [N, CH], [st, ln]])
                    v1 = AP(s, r1s, [[CH * N, 128], [N, CH], [st, ln]])
                    f0 = AP(fw_t[0], ks, [[128, 128], [0, CH], [1, ln]])
                    f1 = AP(fw_t[1], ks, [[128, 128], [0, CH], [1, ln]])
                    vector.tensor_tensor(zdst, v0, f0, mult)
                    last_ins = vector.tensor_tensor(tdst, v1, f1, mult)
                vsync(last_ins)
                for (ks, ln, r0s, r1s, st) in runs:
                    zdst = AP(zt[v], g * CH * N + ks, [[ZGN, 128], [N, CH], [1, ln]])
                    tdst = AP(ztmp, ks, [[CH * N, 128], [N, CH], [1, ln]])
                    last_ins = vector.tensor_add(zdst, zdst, tdst)
                last_ins.then_inc(s_z, 1)

            def combine(r, c):
                gc = r * NC_ + c
                if gc >= 1:
                    vector.wait_ge(s_c, gc)  # WAR mt vs prev combine
                vector.wait_ge(s_g[gc % 2], 16 * (gc // 2 + 1))
                vector.wait_ge(s_wl[gc % 2], 64 * (gc // 2 + 1))
                if gc >= 2:
                    vector.wait_ge(s_o[gc % 2], 16 * (gc // 2))  # WAR accb
                A = At[gc % 2]
                shp = [[GPC * ELEM, 128], [ELEM, GPC], [1, N]]
                oshp = [[GPC * N, 128], [N, GPC], [1, N]]

                def wb(ci):
                    return AP(wt_sb[ci][gc % 2], 0, [[GPC * N, 128], [N, GPC], [1, N]])
                maps = [AP(m, 0, oshp) for m in mt]
                # corner offsets within a gathered element (f16 elems):
                #   vol v: (p0,q0)=v*256, (p1,q0)=v*256+128,
                #          (p0,q1)=v*256+512, (p1,q1)=v*256+640
                last_ins = None
                for v in range(2):
                    b = 4 * v
                    vector.tensor_tensor(maps[b + 0], AP(A, v * 256 + 0, shp), wb(0), mult)
                    vector.tensor_tensor(maps[b + 1], AP(A, v * 256 + 128, shp), wb(2), mult)
                    vector.tensor_tensor(maps[b + 2], AP(A, v * 256 + 512, shp), wb(1), mult)
                    last_ins = vector.tensor_tensor(maps[b + 3], AP(A, v * 256 + 640, shp), wb(3), mult)
                vsync(last_ins)
                for v in range(2):
                    b = 4 * v
                    vector.tensor_add(maps[b + 0], maps[b + 0], maps[b + 1])
                    last_ins = vector.tensor_add(maps[b + 2], maps[b + 2], maps[b + 3])
                vsync(last_ins)
                osh2 = [[GPC * 256, 128], [256, GPC], [1, N]]
                vector.tensor_add(AP(accb[gc % 2], 0, osh2), maps[0], maps[2])
                vector.tensor_add(AP(accb[gc % 2], 128, osh2), maps[4], maps[6]) \
                    .then_inc(s_c, 1)

            for r in range(reps):
                for v in range(2):
                    for g in range(NGRP):
                        zgroup(r, v, g)
                for c in range(NC_):
                    combine(r, c)

    nc.compile()
    return nc


def _exact_label_fixup(label_g, theta, lab_f, out_bool):
    """Recompute voxels of |lab_f - 0.5| < FIX_EPS in the reference's exact
    f32 arithmetic order (validated bit-exact against the jax reference)."""
    eps = np.float32(FIX_EPS)
    cand = np.abs(lab_f - np.float32(0.5)) < eps
    if not cand.any():
        return out_bool
    bb, ii, jj, kk = np.nonzero(cand.reshape(-1, N, N, N))
    v = _exact_reference_values(label_g, theta, bb, ii, jj, kk)
    out_bool.reshape(-1, N, N, N)[bb, ii, jj, kk] = v > np.float32(0.5)
    return out_bool


def _exact_reference_values(vol_g, theta, bb, ii, jj, kk):
    """Reference-order f32 trilinear values at selected voxels.

    Replicates: grid einsum (x*t0 + y*t1 + z*t2, left-assoc f32) + t3; unnorm;
    8-corner accumulation in (z,y,x) order with w=(wz*wy)*wx, out += v*w.
    """
    f32 = np.float32
    t = np.arange(N, dtype=f32)
    xn = ((f32(2.0) * t + f32(1.0)) / f32(N) - f32(1.0)).astype(f32)
    th = theta.astype(f32)

    x = xn[ii]; y = xn[jj]; z = xn[kk]

    # f32 fma via f64 (exact up to negligible double-rounding corner cases)
    def fma32(a, b, c):
        return (np.float64(a) * np.float64(b) + c.astype(np.float64)).astype(f32)

    # grid components — XLA CPU lowers the einsum as an FMA chain (verified
    # bit-exact): fma(z, t2, fma(y, t1, x*t0)) + t3
    def comp(r):
        a = fma32(y, th[r, 1], (x * th[r, 0]).astype(f32))
        a = fma32(z, th[r, 2], a)
        return (a + th[r, 3]).astype(f32)
    gx, gy, gz = comp(0), comp(1), comp(2)

    def unnorm(c):
        return np.clip(((c + f32(1.0)) * f32(N) - f32(1.0)) * f32(0.5), f32(0.0), f32(N - 1))
    ux, uy, uz = unnorm(gx), unnorm(gy), unnorm(gz)
    x0 = np.floor(ux); y0 = np.floor(uy); z0 = np.floor(uz)
    fx = (ux - x0).astype(f32); fy = (uy - y0).astype(f32); fz = (uz - z0).astype(f32)
    x0i = x0.astype(np.int64); y0i = y0.astype(np.int64); z0i = z0.astype(np.int64)
    x1i = np.minimum(x0i + 1, N - 1); y1i = np.minimum(y0i + 1, N - 1); z1i = np.minimum(z0i + 1, N - 1)

    vol = vol_g.reshape(-1, N, N, N)
    out = np.zeros(bb.shape, f32)
    one = f32(1.0)
    for zi, wz in ((z0i, (one - fz).astype(f32)), (z1i, fz)):
        for yi, wy in ((y0i, (one - fy).astype(f32)), (y1i, fy)):
            for xi, wx in ((x0i, (one - fx).astype(f32)), (x1i, fx)):
                # inp[b, c, zi, yi, xi] in transposed space == vol[b, xi, yi, zi]
                vals = vol[bb, xi, yi, zi]
                w = ((wz * wy).astype(f32) * wx).astype(f32)
                out = (out + (vals * w).astype(f32)).astype(f32)
    return out


def _host_fallback(input_g, label_g, transform):
    """Arbitrary-transform fallback: full reference computation on host."""
    bb, ii, jj, kk = np.meshgrid(np.arange(8), np.arange(N), np.arange(N), np.arange(N), indexing="ij")
    bb, ii, jj, kk = (a.reshape(-1) for a in (bb, ii, jj, kk))
    theta = transform[:3].astype(np.float32)
    aug_inp = _exact_reference_values(input_g, theta, bb, ii, jj, kk).reshape(8, 1, N, N, N)
    lab = _exact_reference_values(label_g, theta, bb, ii, jj, kk).reshape(8, 1, N, N, N)
    return aug_inp.astype(np.float32), lab > np.float32(0.5)


def _make_inputs(tables, input_g, label_g):
    idx_p = _pack_idxs(tables["idxA"].reshape(-1))
    # k-replicated per-call weight tiles: wtile[c, call, j, slot*128 + k] =
    # w_c(i = call*8 + slot, j)
    wt = np.empty((4, NCALLS, 128, GPC, N), np.float16)
    for ci, nm in enumerate(("w00", "w01", "w10", "w11")):
        x = tables[nm].T.reshape(128, NCALLS, GPC).astype(np.float16)  # [j, call, slot]
        wt[ci] = x.transpose(1, 0, 2)[:, :, :, None]
    wtile = np.ascontiguousarray(wt.reshape(4, NCALLS, 128, GPC * N))
    fwrep = np.stack([np.tile(1.0 - tables["fw"], (128, 1)),
                      np.tile(tables["fw"], (128, 1))]).astype(np.float16)
    in_maps = []
    for b in range(8):
        in_maps.append({
            # partition-major: vol[p, gc*128+k] = volume[row=gc*128+p, k]
            "vol0": np.ascontiguousarray(
                input_g[b, 0].reshape(128, 128, N).astype(np.float16).transpose(1, 0, 2).reshape(128, NROWS)),
            "vol1": np.ascontiguousarray(
                label_g[b, 0].reshape(128, 128, N).astype(np.float16).transpose(1, 0, 2).reshape(128, NROWS)),
            "idxA": idx_p, "wtile": wtile, "fwrep": fwrep,
        })
    return in_maps


def kernel(input_g, label_g, transform):
    input_g = np.ascontiguousarray(input_g, dtype=np.float32)
    label_g = np.ascontiguousarray(label_g, dtype=np.float32)
    transform = np.asarray(transform, dtype=np.float32)
    theta = transform[:3]

    structured = (abs(float(theta[0, 2])) < 1e-12 and abs(float(theta[1, 2])) < 1e-12
                  and abs(float(theta[2, 0])) < 1e-12 and abs(float(theta[2, 1])) < 1e-12)
    if not structured:
        return _host_fallback(input_g, label_g, transform)

    from concourse.bass_utils import run_bass_kernel_spmd

    tables = _host_tables(theta)
    key = transform.tobytes()
    if key not in _CACHE:
        _CACHE[key] = _build_program(tables)
    nc = _CACHE[key]

    in_maps = _make_inputs(tables, input_g, label_g)
    res = run_bass_kernel_spmd(nc, in_maps, core_ids=list(range(8)))

    aug_inp = np.empty((8, 1, N, N, N), np.float32)
    lab_f = np.empty((8, 1, N, N, N), np.float32)
    for b in range(8):
        oi = res.results[b]["outI"]
        aug_inp[b, 0] = oi[:, 0:128].astype(np.float32).reshape(N, N, N)
        lab_f[b, 0] = oi[:, 128:256].astype(np.float32).reshape(N, N, N)

    out_bool = lab_f > np.float32(0.5)
    out_bool = _exact_label_fixup(label_g, theta, lab_f, out_bool)
    return aug_inp, out_bool
